# revision 20
# baseline (speedup 1.0000x reference)
"""Trainium2 Bass kernel: dual-softmax cross-attention bilinear forms.

Math (per batch b, a = corr[b] in [N, N], N = 3072):
    s_row*s_col = exp(2a) * (1/rowsum) outer (1/colsum),
        rowsum[n] = sum_m exp(a[n,m]),  colsum[m] = sum_n exp(a[n,m])
    fund1 = v1^T attn v1 = X1^T @ (c * v1),  X1 = exp(2a)^T @ (r * v1)
    fund2 = v2^T attn^T v2 -> out2 = (X2^T @ (c * v2)) @ W_proj + b
    out1 = fund1^T @ W_proj + b

Sharding: 8 cores = 4 batches x 2 row-halves; no cross-core traffic.
Each core streams its [1536, 3072] slab (fp16, host-converted) once.
Per 128-row tile: one Exp activation produces E' = exp(a-2) fp16 plus
the row-sums via the activation accumulator; column-sum partials via a
ones-matmul on the PE; E2 = E'^2 on the vector engine stays in SBUF.
The big GEMM X_partial = E2^T @ ((e^2/rowsum) * [v1|v2]) runs on the
tensor engine in fp16, accumulated fp32 in PSUM, exported fp16.

The 12 row tiles are processed in two chunks with separate X outputs so
chunk 2's DMA/exp overlaps chunk 1's GEMM (PE never starves). Host sums
the partial X's in fp32 and applies the small closing contractions.
"""

import numpy as np

import concourse.tile as tile
from concourse import bacc, bass_utils, mybir

B, N, C = 4, 3072, 256
H, W = 48, 64
CP = C + 6          # 262
CP2 = 2 * CP        # 524
NH = N // 2         # 1536 rows per core
NT = NH // 128      # 12 row tiles per core
MT = N // 128       # 24 column tiles
CS_CHUNK = 512
NCS = N // CS_CHUNK  # 6 colsum psum chunks
CHUNKS = ((0, 6), (6, 12))  # row-tile ranges per pipeline chunk

FP32 = mybir.dt.float32
FP16 = mybir.dt.float16
EXP2 = float(np.exp(2.0))

TRACE = False
LAST_RESULT = None
_CACHED_NC = None


def _build_kernel():
    nc = bacc.Bacc("TRN2", target_bir_lowering=False, debug=False)
    a_in = nc.dram_tensor("a_half", [NH, N], FP16, kind="ExternalInput").ap()
    v_in = nc.dram_tensor("v_half", [NH, CP2], FP32, kind="ExternalInput").ap()
    x_outs = [
        nc.dram_tensor(f"x_out{ci}", [N, CP2], FP16, kind="ExternalOutput").ap()
        for ci in range(len(CHUNKS))
    ]
    cs_out = nc.dram_tensor("cs_out", [1, N], FP32, kind="ExternalOutput").ap()

    with tile.TileContext(nc) as tc:
        _kernel_body(tc, a_in, v_in, x_outs, cs_out)
    nc.compile()
    return nc


def _kernel_body(tc, a_in, v_in, x_outs, cs_out):
    nc = tc.nc
    with (
        tc.tile_pool(name="singles", bufs=1) as singles,
        tc.tile_pool(name="a_pool", bufs=5) as a_pool,
        tc.tile_pool(name="e_pool", bufs=2) as e_pool,
        tc.tile_pool(name="e2_pool", bufs=NT) as e2_pool,
        tc.tile_pool(name="x_sb_pool", bufs=4) as x_sb_pool,
        tc.tile_pool(name="cs_psum", bufs=1, space="PSUM") as cs_psum,
        tc.tile_pool(name="x_psum", bufs=3, space="PSUM") as x_psum,
    ):
        ones_t = singles.tile([128, 1], FP16)
        nc.vector.memset(ones_t, 1.0)
        bias_t = singles.tile([128, 1], FP32)
        nc.vector.memset(bias_t, -2.0)

        # prefetch the exp table-set off the critical path
        dummy_t = singles.tile([128, 1], FP32)
        nc.scalar.activation(
            out=dummy_t, in_=bias_t, func=mybir.ActivationFunctionType.Exp
        )

        # all of v in one DMA; per-row stats batched per chunk
        v_sb = singles.tile([128, NT, CP2], FP32)
        nc.sync.dma_start(
            out=v_sb, in_=v_in.rearrange("(t p) c -> p t c", p=128)
        )
        vr_all = singles.tile([128, NT, CP2], FP16)
        rowsum_all = singles.tile([128, NT], FP32)
        rinv_all = singles.tile([128, NT], FP32)

        # 6 colsum chunks packed into 2 psum banks at partitions 0/32/64/96.
        # Banks are pre-zeroed and every matmul accumulates (start=False):
        # correct regardless of has_written state, and sim-safe.
        cs_bank = [
            cs_psum.tile([128, CS_CHUNK], FP32, name=f"csb{t}", tag=f"csb{t}")
            for t in range(2)
        ]
        for t in range(2):
            nc.vector.memset(cs_bank[t], 0.0)

        def cs_ap(j):
            t, p = divmod(j, 4)
            return cs_bank[t][32 * p : 32 * p + 1, :]
        e2_tiles = [None] * NT

        for ci, (i0, i1) in enumerate(CHUNKS):
            # ---- phase 1 (chunk): stream, stats, fp16 exp store ----
            for i in range(i0, i1):
                a_t = a_pool.tile([128, N], FP16)
                nc.sync.dma_start(out=a_t, in_=a_in[i * 128 : (i + 1) * 128, :])

                # E' = exp(a - 2) fp16; rowsum' accumulated per partition
                e_t = e_pool.tile([128, N], FP16)
                nc.scalar.activation(
                    out=e_t,
                    in_=a_t,
                    func=mybir.ActivationFunctionType.Exp,
                    bias=bias_t,
                    scale=1.0,
                    accum_out=rowsum_all[:, i : i + 1],
                )

                # colsum partials: ones^T @ E', accumulated over all tiles
                for j in range(NCS):
                    nc.tensor.matmul(
                        cs_ap(j),
                        lhsT=ones_t,
                        rhs=e_t[:, j * CS_CHUNK : (j + 1) * CS_CHUNK],
                        start=False,
                        stop=(i == NT - 1),
                        skip_group_check=True,
                        tile_position=(0, 32 * (j % 4)),
                    )

                # E2 = E'^2 = exp(2a - 4), fp16, persistent for this chunk
                e2_t = e2_pool.tile([128, N], FP16)
                nc.vector.tensor_mul(e2_t, e_t, e_t)
                e2_tiles[i] = e2_t

            # vr = (e^2 / rowsum) * [v1|v2]  (fp16), stats batched per chunk
            nc.vector.reciprocal(
                rinv_all[:, i0:i1], rowsum_all[:, i0:i1]
            )
            for i in range(i0, i1):
                nc.vector.tensor_scalar(
                    out=vr_all[:, i, :],
                    in0=v_sb[:, i, :],
                    scalar1=rinv_all[:, i : i + 1],
                    scalar2=EXP2,
                    op0=mybir.AluOpType.mult,
                    op1=mybir.AluOpType.mult,
                )

            # ---- phase 2 (chunk): X_chunk = E2_chunk^T @ vr_chunk ----
            for m in range(MT):
                # one [128, 1024] psum tile = 2 banks; matmuls into
                # [0:CP] (bank 0) and [512:512+CP] (bank 1)
                xp = x_psum.tile([128, 1024], FP32)
                for i in range(i0, i1):
                    lhs = e2_tiles[i][:, m * 128 : (m + 1) * 128]
                    nc.tensor.matmul(
                        xp[:, 0:CP], lhsT=lhs, rhs=vr_all[:, i, 0:CP],
                        start=(i == i0), stop=(i == i1 - 1),
                    )
                    nc.tensor.matmul(
                        xp[:, 512 : 512 + CP], lhsT=lhs,
                        rhs=vr_all[:, i, CP:CP2],
                        start=(i == i0), stop=(i == i1 - 1),
                    )
                # merged two-bank copy PSUM -> fp16 SBUF on DVE
                x_sb = x_sb_pool.tile([128, CP2], FP16)
                src = xp.rearrange("p (b x) -> p b x", b=2)[:, :, 0:CP]
                dst = x_sb.rearrange("p (b x) -> p b x", b=2)
                nc.vector.tensor_copy(out=dst, in_=src)
                nc.sync.dma_start(
                    out=x_outs[ci][m * 128 : (m + 1) * 128, :], in_=x_sb
                )

        # colsum psum -> sbuf -> DRAM
        cs_sb = singles.tile([1, N], FP32)
        for j in range(NCS):
            nc.vector.tensor_copy(
                out=cs_sb[:, j * CS_CHUNK : (j + 1) * CS_CHUNK], in_=cs_ap(j)
            )
        nc.sync.dma_start(out=cs_out, in_=cs_sb)


def _positional_encodings():
    ys = np.linspace(-1.0, 1.0, H, dtype=np.float32)
    xs = np.linspace(-1.0, 1.0, W, dtype=np.float32)
    p3 = np.tile(ys, W)
    p4 = np.repeat(xs, H)
    pos = np.stack([p3 * p3, p4 * p4, p3 * p4, p3, p4, np.ones_like(p3)], axis=-1)
    return pos.astype(np.float32)  # [N, 6]


def kernel(x1, x2, corr, W_proj, b_proj):
    global _CACHED_NC, LAST_RESULT
    x1 = np.asarray(x1, dtype=np.float32)
    x2 = np.asarray(x2, dtype=np.float32)
    corr = np.asarray(corr, dtype=np.float32)
    W_proj = np.asarray(W_proj, dtype=np.float32)
    b_proj = np.asarray(b_proj, dtype=np.float32)

    pos = _positional_encodings()
    v1 = np.concatenate([x1, np.broadcast_to(pos, (B, N, 6))], axis=2)  # [B,N,262]
    v2 = np.concatenate([x2, np.broadcast_to(pos, (B, N, 6))], axis=2)
    a = corr.reshape(B, N, N).astype(np.float16)

    if _CACHED_NC is None:
        _CACHED_NC = _build_kernel()
    nc = _CACHED_NC

    in_maps = []
    for b in range(B):
        for h in range(2):
            rows = slice(h * NH, (h + 1) * NH)
            in_maps.append(
                {
                    "a_half": np.ascontiguousarray(a[b, rows, :]),
                    "v_half": np.ascontiguousarray(
                        np.concatenate([v1[b, rows, :], v2[b, rows, :]], axis=1)
                    ),
                }
            )

    res = bass_utils.run_bass_kernel_spmd(
        nc, in_maps, core_ids=list(range(8)), trace=TRACE
    )
    LAST_RESULT = res

    out1 = np.empty((B, CP, C), dtype=np.float32)
    out2 = np.empty((B, CP, C), dtype=np.float32)
    for b in range(B):
        r0, r1 = res.results[2 * b], res.results[2 * b + 1]
        X = np.zeros((N, CP2), dtype=np.float32)
        for r in (r0, r1):
            for ci in range(len(CHUNKS)):
                X += r[f"x_out{ci}"].astype(np.float32)
        colsum = EXP2 * (r0["cs_out"][0] + r1["cs_out"][0])  # [N]
        c = (1.0 / colsum).astype(np.float32)
        vc1 = v1[b] * c[:, None]
        vc2 = v2[b] * c[:, None]
        fund1 = X[:, 0:CP].T @ vc1      # [262, 262] = v1^T attn v1, [c, d]
        fund2t = X[:, CP:CP2].T @ vc2   # = (v2^T attn^T v2)^T, already [d, c]
        out1[b] = fund1.T @ W_proj + b_proj
        out2[b] = fund2t @ W_proj + b_proj
    return (out2, out1)


# revision 21
# speedup vs baseline: 1.0931x; 1.0931x over previous
"""Trainium2 Bass kernel: dual-softmax cross-attention bilinear forms.

Math (per batch b, a = corr[b] in [N, N], N = 3072):
    s_row*s_col = exp(2a) * (1/rowsum) outer (1/colsum),
        rowsum[n] = sum_m exp(a[n,m]),  colsum[m] = sum_n exp(a[n,m])
    fund1 = v1^T attn v1 = X1^T @ (c * v1),  X1 = exp(2a)^T @ (r * v1)
    fund2 = v2^T attn^T v2 -> out2 = (X2^T @ (c * v2)) @ W_proj + b
    out1 = fund1^T @ W_proj + b

Sharding: 8 cores = 4 batches x 2 row-halves; no cross-core traffic.
Each core streams its [1536, 3072] slab (fp16, host-converted) once.
Per 128-row tile: one Exp activation produces E' = exp(a-2) fp16 plus
the row-sums via the activation accumulator; column-sum partials via a
ones-matmul on the PE; E2 = E'^2 on the vector engine stays in SBUF.
The big GEMM X_partial = E2^T @ ((e^2/rowsum) * [v1|v2]) runs on the
tensor engine in fp16, accumulated fp32 in PSUM, exported fp16.

The 12 row tiles are processed in two chunks with separate X outputs so
chunk 2's DMA/exp overlaps chunk 1's GEMM (PE never starves). Host sums
the partial X's in fp32 and applies the small closing contractions.
"""

import numpy as np

import concourse.tile as tile
from concourse import bacc, bass_utils, mybir

B, N, C = 4, 3072, 256
H, W = 48, 64
CP = C + 6          # 262
CP2 = 2 * CP        # 524
NH = N // 2         # 1536 rows per core
NT = NH // 128      # 12 row tiles per core
MT = N // 128       # 24 column tiles
CS_CHUNK = 512
NCS = N // CS_CHUNK  # 6 colsum psum chunks
CHUNKS = ((0, 6), (6, 12))  # row-tile ranges per pipeline chunk

FP32 = mybir.dt.float32
FP16 = mybir.dt.float16
EXP2 = float(np.exp(2.0))

TRACE = False
LAST_RESULT = None
_CACHED_NC = None


def _build_kernel():
    nc = bacc.Bacc("TRN2", target_bir_lowering=False, debug=False)
    a_in = nc.dram_tensor("a_half", [NH, N], FP16, kind="ExternalInput").ap()
    v_in = nc.dram_tensor("v_half", [NH, CP2], FP32, kind="ExternalInput").ap()
    x_outs = [
        nc.dram_tensor(f"x_out{ci}", [N, CP2], FP16, kind="ExternalOutput").ap()
        for ci in range(len(CHUNKS))
    ]
    cs_out = nc.dram_tensor("cs_out", [1, N], FP32, kind="ExternalOutput").ap()

    with tile.TileContext(nc) as tc:
        _kernel_body(tc, a_in, v_in, x_outs, cs_out)
    nc.compile()
    return nc


def _kernel_body(tc, a_in, v_in, x_outs, cs_out):
    nc = tc.nc
    with (
        tc.tile_pool(name="singles", bufs=1) as singles,
        tc.tile_pool(name="a_pool", bufs=5) as a_pool,
        tc.tile_pool(name="e_pool", bufs=2) as e_pool,
        tc.tile_pool(name="e2_pool", bufs=NT) as e2_pool,
        tc.tile_pool(name="x_sb_pool", bufs=4) as x_sb_pool,
        tc.tile_pool(name="cs_psum", bufs=1, space="PSUM") as cs_psum,
        tc.tile_pool(name="x_psum", bufs=3, space="PSUM") as x_psum,
    ):
        ones_t = singles.tile([128, 1], FP16)
        nc.vector.memset(ones_t, 1.0)
        bias_t = singles.tile([128, 1], FP32)
        nc.vector.memset(bias_t, -2.0)

        # prefetch the exp table-set off the critical path
        dummy_t = singles.tile([128, 1], FP32)
        nc.scalar.activation(
            out=dummy_t, in_=bias_t, func=mybir.ActivationFunctionType.Exp
        )

        # all of v in one DMA; per-row stats batched per chunk
        v_sb = singles.tile([128, NT, CP2], FP32)
        nc.sync.dma_start(
            out=v_sb, in_=v_in.rearrange("(t p) c -> p t c", p=128)
        )
        vr_all = singles.tile([128, NT, CP2], FP16)
        rowsum_all = singles.tile([128, NT], FP32)
        rinv_all = singles.tile([128, NT], FP32)

        # 6 colsum chunks packed into 2 psum banks at partitions 0/32/64/96.
        # Banks are pre-zeroed and every matmul accumulates (start=False):
        # correct regardless of has_written state, and sim-safe.
        cs_bank = [
            cs_psum.tile([128, CS_CHUNK], FP32, name=f"csb{t}", tag=f"csb{t}")
            for t in range(2)
        ]
        for t in range(2):
            nc.vector.memset(cs_bank[t], 0.0)

        def cs_ap(j):
            t, p = divmod(j, 4)
            return cs_bank[t][32 * p : 32 * p + 1, :]
        e2_tiles = [None] * NT

        for ci, (i0, i1) in enumerate(CHUNKS):
            # ---- phase 1 (chunk): stream, stats, fp16 exp store ----
            for i in range(i0, i1):
                a_t = a_pool.tile([128, N], FP16)
                nc.sync.dma_start(out=a_t, in_=a_in[i * 128 : (i + 1) * 128, :])

                # E' = exp(a - 2) fp16; rowsum' accumulated per partition
                e_t = e_pool.tile([128, N], FP16)
                nc.scalar.activation(
                    out=e_t,
                    in_=a_t,
                    func=mybir.ActivationFunctionType.Exp,
                    bias=bias_t,
                    scale=1.0,
                    accum_out=rowsum_all[:, i : i + 1],
                )

                # colsum partials: ones^T @ E', accumulated over all tiles
                for j in range(NCS):
                    nc.tensor.matmul(
                        cs_ap(j),
                        lhsT=ones_t,
                        rhs=e_t[:, j * CS_CHUNK : (j + 1) * CS_CHUNK],
                        start=False,
                        stop=(i == NT - 1),
                        skip_group_check=True,
                        tile_position=(0, 32 * (j % 4)),
                    )

                # vr = (e^2 / rowsum) * [v1|v2]  (fp16)
                nc.vector.reciprocal(
                    rinv_all[:, i : i + 1], rowsum_all[:, i : i + 1]
                )
                nc.vector.tensor_scalar(
                    out=vr_all[:, i, :],
                    in0=v_sb[:, i, :],
                    scalar1=rinv_all[:, i : i + 1],
                    scalar2=EXP2,
                    op0=mybir.AluOpType.mult,
                    op1=mybir.AluOpType.mult,
                )

                # E2 = E'^2 = exp(2a - 4), fp16, persistent for this chunk
                e2_t = e2_pool.tile([128, N], FP16)
                nc.vector.tensor_mul(e2_t, e_t, e_t)
                e2_tiles[i] = e2_t

            # ---- phase 2 (chunk): X_chunk = E2_chunk^T @ vr_chunk ----
            for m in range(MT):
                # one [128, 1024] psum tile = 2 banks; matmuls into
                # [0:CP] (bank 0) and [512:512+CP] (bank 1)
                xp = x_psum.tile([128, 1024], FP32)
                for i in range(i0, i1):
                    lhs = e2_tiles[i][:, m * 128 : (m + 1) * 128]
                    nc.tensor.matmul(
                        xp[:, 0:CP], lhsT=lhs, rhs=vr_all[:, i, 0:CP],
                        start=(i == i0), stop=(i == i1 - 1),
                    )
                    nc.tensor.matmul(
                        xp[:, 512 : 512 + CP], lhsT=lhs,
                        rhs=vr_all[:, i, CP:CP2],
                        start=(i == i0), stop=(i == i1 - 1),
                    )
                # merged two-bank copy PSUM -> fp16 SBUF on DVE
                x_sb = x_sb_pool.tile([128, CP2], FP16)
                src = xp.rearrange("p (b x) -> p b x", b=2)[:, :, 0:CP]
                dst = x_sb.rearrange("p (b x) -> p b x", b=2)
                nc.vector.tensor_copy(out=dst, in_=src)
                nc.sync.dma_start(
                    out=x_outs[ci][m * 128 : (m + 1) * 128, :], in_=x_sb
                )

        # colsum psum -> sbuf -> DRAM
        cs_sb = singles.tile([1, N], FP32)
        for j in range(NCS):
            nc.vector.tensor_copy(
                out=cs_sb[:, j * CS_CHUNK : (j + 1) * CS_CHUNK], in_=cs_ap(j)
            )
        nc.sync.dma_start(out=cs_out, in_=cs_sb)


def _positional_encodings():
    ys = np.linspace(-1.0, 1.0, H, dtype=np.float32)
    xs = np.linspace(-1.0, 1.0, W, dtype=np.float32)
    p3 = np.tile(ys, W)
    p4 = np.repeat(xs, H)
    pos = np.stack([p3 * p3, p4 * p4, p3 * p4, p3, p4, np.ones_like(p3)], axis=-1)
    return pos.astype(np.float32)  # [N, 6]


def kernel(x1, x2, corr, W_proj, b_proj):
    global _CACHED_NC, LAST_RESULT
    x1 = np.asarray(x1, dtype=np.float32)
    x2 = np.asarray(x2, dtype=np.float32)
    corr = np.asarray(corr, dtype=np.float32)
    W_proj = np.asarray(W_proj, dtype=np.float32)
    b_proj = np.asarray(b_proj, dtype=np.float32)

    pos = _positional_encodings()
    v1 = np.concatenate([x1, np.broadcast_to(pos, (B, N, 6))], axis=2)  # [B,N,262]
    v2 = np.concatenate([x2, np.broadcast_to(pos, (B, N, 6))], axis=2)
    a = corr.reshape(B, N, N).astype(np.float16)

    if _CACHED_NC is None:
        _CACHED_NC = _build_kernel()
    nc = _CACHED_NC

    in_maps = []
    for b in range(B):
        for h in range(2):
            rows = slice(h * NH, (h + 1) * NH)
            in_maps.append(
                {
                    "a_half": np.ascontiguousarray(a[b, rows, :]),
                    "v_half": np.ascontiguousarray(
                        np.concatenate([v1[b, rows, :], v2[b, rows, :]], axis=1)
                    ),
                }
            )

    res = bass_utils.run_bass_kernel_spmd(
        nc, in_maps, core_ids=list(range(8)), trace=TRACE
    )
    LAST_RESULT = res

    out1 = np.empty((B, CP, C), dtype=np.float32)
    out2 = np.empty((B, CP, C), dtype=np.float32)
    for b in range(B):
        r0, r1 = res.results[2 * b], res.results[2 * b + 1]
        X = np.zeros((N, CP2), dtype=np.float32)
        for r in (r0, r1):
            for ci in range(len(CHUNKS)):
                X += r[f"x_out{ci}"].astype(np.float32)
        colsum = EXP2 * (r0["cs_out"][0] + r1["cs_out"][0])  # [N]
        c = (1.0 / colsum).astype(np.float32)
        vc1 = v1[b] * c[:, None]
        vc2 = v2[b] * c[:, None]
        fund1 = X[:, 0:CP].T @ vc1      # [262, 262] = v1^T attn v1, [c, d]
        fund2t = X[:, CP:CP2].T @ vc2   # = (v2^T attn^T v2)^T, already [d, c]
        out1[b] = fund1.T @ W_proj + b_proj
        out2[b] = fund2t @ W_proj + b_proj
    return (out2, out1)
